# revision 5
# baseline (speedup 1.0000x reference)
"""Trainium2 Bass kernel for nn_LocalRNN (local GRU, chunked scan).

Problem: B=32, S=2048, I=H=256, ksize=16. Each ksize-chunk runs a GRU from
h0=0, so the 32*128=4096 chunks are independent length-16 GRU chains.

Sharding: data-parallel over chunks — core c gets batch rows [4c:4c+4],
i.e. 512 chains. Weights replicated.

Per-core kernel layout ("transposed"): gate/hidden dim on partitions, chain
(seq) index on the free dim (N=512 chains, one group). Per step t:

  gates[3H, 512] = W_ih @ x_t^T + W_hh @ h_{t-1}^T       (PSUM accumulation)
  r = sigmoid(psum_r + (b_ih+b_hh)_r)                    (ScalarE, bias port)
  z = sigmoid(psum_z + (b_ih+b_hh)_z)
  n = tanh((psum_in + b_ih_n) + r*(psum_hn + b_hn))      (fused DVE stt ops)
  h = n + z*(h_prev - n)          (d=h-n on GpSimd, e/h on VectorE, per half)

The x-side and h-side matmuls for r/z accumulate into the same PSUM bank so
no explicit adds are needed; n keeps separate x/h banks because r multiplies
only the h side. PSUM: 4 gate tensors x 2 halves x [128,512]f32 = all 8
banks; banks free progressively (r bank after sigmoid_r etc.) so step t+1
x-side matmuls overlap step t's elementwise tail.

Matmul operands and SBUF elementwise tensors are fp16 (PE fast-weight-load +
DVE 2x mode, ~8x finer mantissa than bf16; values are O(1) so fp16 range is
safe); PSUM accumulation is fp32. Host pre-transposes x / weights into
DMA-friendly contiguous blocks and inverts the output layout at the end.
"""

import sys

for _p in ("/opt/trn_rl_repo", "/root/.axon_site"):
    if _p not in sys.path:
        sys.path.insert(0, _p)

import numpy as np

import concourse.bass as bass  # noqa: F401
import concourse.tile as tile
from concourse import bacc, mybir
from concourse.bass_utils import run_bass_kernel_spmd

# Problem constants (hardcoded per harness contract).
B, S, I, H = 32, 2048, 256, 256
KSIZE = 16
NCORES = 8
ROWS_PER_CORE = B // NCORES            # 4 batch rows per core
CHUNKS_PER_ROW = S // KSIZE            # 128
SEQS = ROWS_PER_CORE * CHUNKS_PER_ROW  # 512 chains per core
NS = SEQS                              # free-dim width per op
KT = 2                                 # contraction tiles (I/128 = H/128 = 2)

F32 = mybir.dt.float32
F16 = mybir.dt.float16
AF = mybir.ActivationFunctionType
OP = mybir.AluOpType

MM_DT = F16
NP_MM_DT = np.float16


def build_nc():
    nc = bacc.Bacc("TRN2", target_bir_lowering=False, debug=False)

    # Inputs (host pre-transposed, contiguous per-DMA blocks).
    # xt[t, p, k, s] = x_shard[seq=s, t, i=k*128+p]
    xt_d = nc.dram_tensor("xt", [KSIZE, 128, KT, NS], MM_DT, kind="ExternalInput")
    # wih_t[p, k, m] = W_ih[m, k*128+p]  (transposed weight, lhsT layout)
    wih_d = nc.dram_tensor("wih_t", [128, KT, 3 * H], MM_DT, kind="ExternalInput")
    whh_d = nc.dram_tensor("whh_t", [128, KT, 3 * H], MM_DT, kind="ExternalInput")
    # brz[p, mi] = (b_ih+b_hh)[mi*128+p] for mi in 0..3 (r0,r1,z0,z1)
    brz_d = nc.dram_tensor("brz", [128, 4], F32, kind="ExternalInput")
    # bhn[p, m] = b_hh[2H + m*128 + p]; bin[p, m] = b_ih[2H + m*128 + p]
    bhn_d = nc.dram_tensor("bhn", [128, 2], F32, kind="ExternalInput")
    bin_d = nc.dram_tensor("bin", [128, 2], F32, kind="ExternalInput")
    # out[t, p, m, s] = h_t[seq=s, hdim=m*128+p]
    out_d = nc.dram_tensor("out", [KSIZE, 128, 2, NS], MM_DT, kind="ExternalOutput")

    with tile.TileContext(nc) as tc:
        with (
            tc.tile_pool(name="consts", bufs=1) as consts,
            tc.tile_pool(name="xp", bufs=4) as xp,
            tc.tile_pool(name="ps", bufs=1, space="PSUM") as ps,
            tc.tile_pool(name="work", bufs=2) as work,
            tc.tile_pool(name="hp", bufs=2) as hp,
        ):
            wih = consts.tile([128, KT, 3 * H], MM_DT)
            nc.sync.dma_start(wih[:], wih_d.ap())
            whh = consts.tile([128, KT, 3 * H], MM_DT)
            nc.sync.dma_start(whh[:], whh_d.ap())
            brz = consts.tile([128, 4], F32)
            nc.sync.dma_start(brz[:], brz_d.ap())
            bhn = consts.tile([128, 2], F32)
            nc.sync.dma_start(bhn[:], bhn_d.ap())
            bin_ = consts.tile([128, 2], F32)
            nc.sync.dma_start(bin_[:], bin_d.ap())

            h_state = None
            for t in range(KSIZE):
                xs = xp.tile([128, KT, NS], MM_DT, tag="x")
                nc.sync.dma_start(xs[:], xt_d.ap()[t])
                xr = xs[:]
                hr = None if t == 0 else h_state[:]

                # PSUM: each [:, m, :] slice is one full 2KB bank.
                bank_r = ps.tile([128, 2, NS], F32, tag="r")
                bank_z = ps.tile([128, 2, NS], F32, tag="z")
                bank_in = ps.tile([128, 2, NS], F32, tag="in")
                bank_hn = None if t == 0 else ps.tile([128, 2, NS], F32, tag="hn")

                # Matmuls. W row tiles: r halves mi=0,1; z mi=2,3; n mi=4,5.
                # Emission order: r first (its sigmoid leads the chain), then
                # hn (feeds tmp), then z / in (consumed later).
                def mm_gate(bank_t, mi_base, m, x_only):
                    col = slice((mi_base + m) * 128, (mi_base + m + 1) * 128)
                    n_mm = KT if (t == 0 or x_only == "x" or x_only == "h") else 2 * KT
                    i_mm = 0
                    if x_only in ("x", "both"):
                        for k in range(KT):
                            nc.tensor.matmul(
                                bank_t[:, m, :], wih[:, k, col], xr[:, k, :],
                                start=(i_mm == 0), stop=(i_mm == n_mm - 1),
                            )
                            i_mm += 1
                    if x_only in ("h", "both") and t > 0:
                        for k in range(KT):
                            nc.tensor.matmul(
                                bank_t[:, m, :], whh[:, k, col], hr[:, k, :],
                                start=(i_mm == 0 and x_only == "h"),
                                stop=(i_mm == n_mm - 1),
                            )
                            i_mm += 1

                for m in range(2):
                    mm_gate(bank_r, 0, m, "both" if t > 0 else "x")
                if t > 0:
                    for m in range(2):
                        mm_gate(bank_hn, 4, m, "h")
                for m in range(2):
                    mm_gate(bank_z, 2, m, "both" if t > 0 else "x")
                for m in range(2):
                    mm_gate(bank_in, 4, m, "x")

                # Elementwise (per gate-half [128, NS]).
                rz = work.tile([128, 4, NS], MM_DT, tag="rz")
                for mi in (0, 1, 2, 3):  # r0, r1, z0, z1
                    bank_t = bank_r if mi < 2 else bank_z
                    nc.scalar.activation(
                        rz[:, mi, :], bank_t[:, mi % 2, :], AF.Sigmoid,
                        bias=brz[:, mi : mi + 1],
                    )

                tmp = work.tile([128, 2, NS], MM_DT, tag="tmp")
                pren = work.tile([128, 2, NS], MM_DT, tag="pren")
                n_t = work.tile([128, 2, NS], MM_DT, tag="n")
                hnew = hp.tile([128, 2, NS], MM_DT, tag="h")
                e = work.tile([128, 2, NS], MM_DT, tag="e")
                d = work.tile([128, 2, NS], MM_DT, tag="d")
                for m in range(2):
                    if t == 0:
                        nc.vector.tensor_scalar_mul(
                            tmp[:, m, :], rz[:, m, :], bhn[:, m : m + 1]
                        )
                    else:
                        # tmp = (psum_hn + b_hh_n) * r
                        nc.vector.scalar_tensor_tensor(
                            tmp[:, m, :], bank_hn[:, m, :], bhn[:, m : m + 1],
                            rz[:, m, :], op0=OP.add, op1=OP.mult,
                        )
                    # pre_n = (psum_in + b_ih_n) + tmp
                    nc.vector.scalar_tensor_tensor(
                        pren[:, m, :], bank_in[:, m, :], bin_[:, m : m + 1],
                        tmp[:, m, :], op0=OP.add, op1=OP.add,
                    )
                    nc.scalar.activation(n_t[:, m, :], pren[:, m, :], AF.Tanh)
                    if t == 0:
                        # h1 = n - z*n
                        nc.vector.tensor_tensor(
                            e[:, m, :], rz[:, 2 + m, :], n_t[:, m, :], op=OP.mult
                        )
                        nc.vector.tensor_tensor(
                            hnew[:, m, :], n_t[:, m, :], e[:, m, :], op=OP.subtract
                        )
                    else:
                        # h = n + z*(h_prev - n); d on GpSimd (idle engine)
                        nc.gpsimd.tensor_tensor(
                            d[:, m, :], h_state[:, m, :], n_t[:, m, :],
                            op=OP.subtract,
                        )
                        nc.vector.tensor_tensor(
                            e[:, m, :], rz[:, 2 + m, :], d[:, m, :], op=OP.mult
                        )
                        nc.vector.tensor_tensor(
                            hnew[:, m, :], e[:, m, :], n_t[:, m, :], op=OP.add
                        )

                nc.sync.dma_start(out_d.ap()[t], hnew[:])
                h_state = hnew

    nc.compile()
    return nc


_NC_CACHE = None


def _get_nc():
    global _NC_CACHE
    if _NC_CACHE is None:
        _NC_CACHE = build_nc()
    return _NC_CACHE


def _prep_shared(W_ih, W_hh, b_ih, b_hh):
    wih_t = np.ascontiguousarray(
        W_ih.T.reshape(KT, 128, 3 * H).transpose(1, 0, 2)
    ).astype(NP_MM_DT)
    whh_t = np.ascontiguousarray(
        W_hh.T.reshape(KT, 128, 3 * H).transpose(1, 0, 2)
    ).astype(NP_MM_DT)
    bsum = b_ih + b_hh
    brz = np.ascontiguousarray(bsum[: 2 * H].reshape(4, 128).T)
    bhn = np.ascontiguousarray(b_hh[2 * H :].reshape(2, 128).T)
    bin_ = np.ascontiguousarray(b_ih[2 * H :].reshape(2, 128).T)
    return wih_t, whh_t, brz, bhn, bin_


def _prep_core_inputs(x, shared, core):
    wih_t, whh_t, brz, bhn, bin_ = shared
    xc = x[core * ROWS_PER_CORE : (core + 1) * ROWS_PER_CORE]  # [4, S, I]
    xc = xc.reshape(SEQS, KSIZE, I)
    # xt[t, p, k, s] = xc[s, t, k*128+p]
    xt = np.ascontiguousarray(
        xc.reshape(NS, KSIZE, KT, 128).transpose(1, 3, 2, 0)
    ).astype(NP_MM_DT)
    return {
        "xt": xt,
        "wih_t": wih_t,
        "whh_t": whh_t,
        "brz": brz,
        "bhn": bhn,
        "bin": bin_,
    }


def kernel(x, W_ih, W_hh, b_ih, b_hh, ksize):
    x = np.asarray(x, dtype=np.float32)
    W_ih = np.asarray(W_ih, dtype=np.float32)
    W_hh = np.asarray(W_hh, dtype=np.float32)
    b_ih = np.asarray(b_ih, dtype=np.float32)
    b_hh = np.asarray(b_hh, dtype=np.float32)
    assert int(ksize) == KSIZE and x.shape == (B, S, I)

    shared = _prep_shared(W_ih, W_hh, b_ih, b_hh)
    in_maps = [_prep_core_inputs(x, shared, c) for c in range(NCORES)]
    nc = _get_nc()
    res = run_bass_kernel_spmd(nc, in_maps, core_ids=list(range(NCORES)))

    out = np.empty((B, S, H), dtype=np.float32)
    for c in range(NCORES):
        oc = np.asarray(res.results[c]["out"]).astype(np.float32)  # [t,p,m,s]
        # h[seq=s, t, hdim=m*128+p]
        hc = oc.transpose(3, 0, 2, 1).reshape(SEQS, KSIZE, H)
        out[c * ROWS_PER_CORE : (c + 1) * ROWS_PER_CORE] = hc.reshape(
            ROWS_PER_CORE, S, H
        )
    return out


# revision 6
# speedup vs baseline: 1.4112x; 1.4112x over previous
"""Trainium2 Bass kernel for nn_LocalRNN (local GRU, chunked scan).

Problem: B=32, S=2048, I=H=256, ksize=16. Each ksize-chunk runs a GRU from
h0=0, so the 32*128=4096 chunks are independent length-16 GRU chains.

Sharding: data-parallel over chunks — core c gets batch rows [4c:4c+4],
i.e. 512 chains. Weights replicated.

Per-core kernel layout ("transposed"): gate/hidden dim on partitions, chain
(seq) index on the free dim. Per step t and seq-group g (2 groups x 256 seqs):

  gates[3H, seqs] = W_ih @ x_t^T + W_hh @ h_{t-1}^T     (PSUM accumulation)
  r = sigmoid(psum_r + (b_ih+b_hh)_r)                    (ScalarE, bias port)
  z = sigmoid(psum_z + (b_ih+b_hh)_z)
  n = tanh((psum_in + b_ih_n) + r*(psum_hn + b_hh_n))    (fused DVE stt ops)
  h = n + z*(h_prev - n)

The x-side and h-side matmuls for r/z accumulate into the same PSUM bank so
no explicit adds are needed; n keeps separate x/h banks because r multiplies
only the h side. PSUM budget: 4 banks per group x 2 groups = all 8 banks,
ping-ponged so one group's matmuls overlap the other group's elementwise.

Matmul operands and SBUF elementwise tensors are fp16 (PE fast-weight-load +
DVE 2x mode, ~8x finer mantissa than bf16; values are O(1) so fp16 range is
safe); PSUM accumulation is fp32. Host pre-transposes x / weights into
DMA-friendly contiguous blocks and inverts the output layout at the end.
"""

import sys

for _p in ("/opt/trn_rl_repo", "/root/.axon_site"):
    if _p not in sys.path:
        sys.path.insert(0, _p)

import ml_dtypes
import numpy as np

import concourse.bass as bass  # noqa: F401
import concourse.tile as tile
from concourse import bacc, mybir
from concourse.bass_utils import run_bass_kernel_spmd

# Problem constants (hardcoded per harness contract).
B, S, I, H = 32, 2048, 256, 256
KSIZE = 16
NCORES = 8
ROWS_PER_CORE = B // NCORES            # 4 batch rows per core
CHUNKS_PER_ROW = S // KSIZE            # 128
SEQS = ROWS_PER_CORE * CHUNKS_PER_ROW  # 512 chains per core
G = 2                                  # seq groups per core
NS = SEQS // G                         # 256 seqs per group
KT = 2                                 # contraction tiles (I/128 = H/128 = 2)

F32 = mybir.dt.float32
F16 = mybir.dt.float16
AF = mybir.ActivationFunctionType
OP = mybir.AluOpType

MM_DT = F16         # matmul operand + elementwise SBUF dtype
NP_MM_DT = np.float16


def build_nc():
    nc = bacc.Bacc("TRN2", target_bir_lowering=False, debug=False)

    # Inputs (host pre-transposed, contiguous per-DMA blocks).
    # xt[t, g, p, k, s] = x_shard[seq=g*NS+s, t, i=k*128+p]
    xt_d = nc.dram_tensor("xt", [KSIZE, G, 128, KT, NS], MM_DT, kind="ExternalInput")
    # wih_t[p, k, m] = W_ih[m, k*128+p]  (transposed weight, lhsT layout)
    wih_d = nc.dram_tensor("wih_t", [128, KT, 3 * H], MM_DT, kind="ExternalInput")
    whh_d = nc.dram_tensor("whh_t", [128, KT, 3 * H], MM_DT, kind="ExternalInput")
    # brz[p, mi] = (b_ih+b_hh)[mi*128+p] for mi in 0..3 (r0,r1,z0,z1)
    brz_d = nc.dram_tensor("brz", [128, 4], F32, kind="ExternalInput")
    # bhn[p, m] = b_hh[2H + m*128 + p]; bin[p, m] = b_ih[2H + m*128 + p]
    bhn_d = nc.dram_tensor("bhn", [128, 2], F32, kind="ExternalInput")
    bin_d = nc.dram_tensor("bin", [128, 2], F32, kind="ExternalInput")
    # out[t, g, p, m, s] = h_t[seq=g*NS+s, hdim=m*128+p]
    out_d = nc.dram_tensor("out", [KSIZE, G, 128, 2, NS], MM_DT, kind="ExternalOutput")

    with tile.TileContext(nc) as tc:
        with (
            tc.tile_pool(name="consts", bufs=1) as consts,
            tc.tile_pool(name="xp", bufs=8) as xp,
            tc.tile_pool(name="ps", bufs=2, space="PSUM") as ps,
            tc.tile_pool(name="work", bufs=4) as work,
            tc.tile_pool(name="hp", bufs=4) as hp,
        ):
            wih = consts.tile([128, KT, 3 * H], MM_DT)
            nc.sync.dma_start(wih[:], wih_d.ap())
            whh = consts.tile([128, KT, 3 * H], MM_DT)
            nc.sync.dma_start(whh[:], whh_d.ap())
            brz = consts.tile([128, 4], F32)
            nc.sync.dma_start(brz[:], brz_d.ap())
            bhn = consts.tile([128, 2], F32)
            nc.sync.dma_start(bhn[:], bhn_d.ap())
            bin_ = consts.tile([128, 2], F32)
            nc.sync.dma_start(bin_[:], bin_d.ap())

            h_state = [None] * G
            for t in range(KSIZE):
                for g in range(G):
                    xs = xp.tile([128, KT, NS], MM_DT, tag="x")
                    nc.sync.dma_start(xs[:], xt_d.ap()[t, g])
                    xr = xs[:]
                    hr = None if t == 0 else h_state[g][:]

                    # PSUM banks: [128, 2, NS] f32 = one 2KB bank each.
                    bank_r = ps.tile([128, 2, NS], F32, tag="r")
                    bank_z = ps.tile([128, 2, NS], F32, tag="z")
                    bank_in = ps.tile([128, 2, NS], F32, tag="in")
                    bank_hn = None if t == 0 else ps.tile([128, 2, NS], F32, tag="hn")

                    # Matmuls. W row tiles: r halves mi=0,1; z mi=2,3; n mi=4,5.
                    for m in range(2):
                        for gate, bank_t in (("r", bank_r), ("z", bank_z)):
                            mi = m if gate == "r" else 2 + m
                            col = slice(mi * 128, (mi + 1) * 128)
                            n_mm = KT if t == 0 else 2 * KT
                            i_mm = 0
                            for k in range(KT):
                                nc.tensor.matmul(
                                    bank_t[:, m, :], wih[:, k, col], xr[:, k, :],
                                    start=(i_mm == 0), stop=(i_mm == n_mm - 1),
                                )
                                i_mm += 1
                            if t > 0:
                                for k in range(KT):
                                    nc.tensor.matmul(
                                        bank_t[:, m, :], whh[:, k, col], hr[:, k, :],
                                        start=False, stop=(i_mm == n_mm - 1),
                                    )
                                    i_mm += 1
                        # n gate: x-side into bank_in, h-side into bank_hn.
                        col = slice((4 + m) * 128, (5 + m) * 128)
                        for k in range(KT):
                            nc.tensor.matmul(
                                bank_in[:, m, :], wih[:, k, col], xr[:, k, :],
                                start=(k == 0), stop=(k == KT - 1),
                            )
                        if t > 0:
                            for k in range(KT):
                                nc.tensor.matmul(
                                    bank_hn[:, m, :], whh[:, k, col], hr[:, k, :],
                                    start=(k == 0), stop=(k == KT - 1),
                                )

                    # Elementwise.
                    rz = work.tile([128, 4, NS], MM_DT, tag="rz")
                    for mi in range(2):
                        nc.scalar.activation(
                            rz[:, mi, :], bank_r[:, mi, :], AF.Sigmoid,
                            bias=brz[:, mi : mi + 1],
                        )
                        nc.scalar.activation(
                            rz[:, 2 + mi, :], bank_z[:, mi, :], AF.Sigmoid,
                            bias=brz[:, 2 + mi : 3 + mi],
                        )

                    tmp = work.tile([128, 2, NS], MM_DT, tag="tmp")
                    pren = work.tile([128, 2, NS], MM_DT, tag="pren")
                    for m in range(2):
                        if t == 0:
                            # h=0: h-side n contribution is just b_hh_n.
                            nc.vector.tensor_scalar_mul(
                                tmp[:, m, :], rz[:, m, :], bhn[:, m : m + 1]
                            )
                        else:
                            # tmp = (psum_hn + b_hh_n) * r
                            nc.vector.scalar_tensor_tensor(
                                tmp[:, m, :], bank_hn[:, m, :], bhn[:, m : m + 1],
                                rz[:, m, :], op0=OP.add, op1=OP.mult,
                            )
                        # pre_n = (psum_in + b_ih_n) + tmp
                        nc.vector.scalar_tensor_tensor(
                            pren[:, m, :], bank_in[:, m, :], bin_[:, m : m + 1],
                            tmp[:, m, :], op0=OP.add, op1=OP.add,
                        )

                    n_t = work.tile([128, 2, NS], MM_DT, tag="n")
                    nc.scalar.activation(n_t[:], pren[:], AF.Tanh)

                    hnew = hp.tile([128, 2, NS], MM_DT, tag="h")
                    e = work.tile([128, 2, NS], MM_DT, tag="e")
                    if t == 0:
                        # h1 = n - z*n
                        nc.vector.tensor_tensor(e[:], rz[:, 2:4, :], n_t[:], op=OP.mult)
                        nc.vector.tensor_tensor(hnew[:], n_t[:], e[:], op=OP.subtract)
                    else:
                        d = work.tile([128, 2, NS], MM_DT, tag="d")
                        # h = n + z*(h_prev - n)
                        nc.vector.tensor_tensor(
                            d[:], h_state[g][:], n_t[:], op=OP.subtract
                        )
                        nc.vector.tensor_tensor(e[:], rz[:, 2:4, :], d[:], op=OP.mult)
                        nc.vector.tensor_tensor(hnew[:], e[:], n_t[:], op=OP.add)

                    nc.sync.dma_start(out_d.ap()[t, g], hnew[:])
                    h_state[g] = hnew

    nc.compile()
    return nc


_NC_CACHE = None


def _get_nc():
    global _NC_CACHE
    if _NC_CACHE is None:
        _NC_CACHE = build_nc()
    return _NC_CACHE


def _prep_shared(W_ih, W_hh, b_ih, b_hh):
    wih_t = np.ascontiguousarray(
        W_ih.T.reshape(KT, 128, 3 * H).transpose(1, 0, 2)
    ).astype(NP_MM_DT)
    whh_t = np.ascontiguousarray(
        W_hh.T.reshape(KT, 128, 3 * H).transpose(1, 0, 2)
    ).astype(NP_MM_DT)
    bsum = b_ih + b_hh
    brz = np.ascontiguousarray(bsum[: 2 * H].reshape(4, 128).T)
    bhn = np.ascontiguousarray(b_hh[2 * H :].reshape(2, 128).T)
    bin_ = np.ascontiguousarray(b_ih[2 * H :].reshape(2, 128).T)
    return wih_t, whh_t, brz, bhn, bin_


def _prep_core_inputs(x, shared, core):
    wih_t, whh_t, brz, bhn, bin_ = shared
    xc = x[core * ROWS_PER_CORE : (core + 1) * ROWS_PER_CORE]  # [4, S, I]
    xc = xc.reshape(SEQS, KSIZE, I)
    # xt[t, g, p, k, s] = xc[g*NS+s, t, k*128+p]
    xt = np.ascontiguousarray(
        xc.reshape(G, NS, KSIZE, KT, 128).transpose(2, 0, 4, 3, 1)
    ).astype(NP_MM_DT)
    return {
        "xt": xt,
        "wih_t": wih_t,
        "whh_t": whh_t,
        "brz": brz,
        "bhn": bhn,
        "bin": bin_,
    }


def kernel(x, W_ih, W_hh, b_ih, b_hh, ksize):
    x = np.asarray(x, dtype=np.float32)
    W_ih = np.asarray(W_ih, dtype=np.float32)
    W_hh = np.asarray(W_hh, dtype=np.float32)
    b_ih = np.asarray(b_ih, dtype=np.float32)
    b_hh = np.asarray(b_hh, dtype=np.float32)
    assert int(ksize) == KSIZE and x.shape == (B, S, I)

    shared = _prep_shared(W_ih, W_hh, b_ih, b_hh)
    in_maps = [_prep_core_inputs(x, shared, c) for c in range(NCORES)]
    nc = _get_nc()
    res = run_bass_kernel_spmd(nc, in_maps, core_ids=list(range(NCORES)))

    out = np.empty((B, S, H), dtype=np.float32)
    for c in range(NCORES):
        oc = np.asarray(res.results[c]["out"]).astype(np.float32)  # [t,g,p,m,s]
        # h[seq=g*NS+s, t, hdim=m*128+p]
        hc = oc.transpose(1, 4, 0, 3, 2).reshape(SEQS, KSIZE, H)
        out[c * ROWS_PER_CORE : (c + 1) * ROWS_PER_CORE] = hc.reshape(
            ROWS_PER_CORE, S, H
        )
    return out
